# revision 6
# baseline (speedup 1.0000x reference)
"""Trainium2 Bass kernel for CrossDepthAttentionResidual (v2, bf16 pipeline).

Reference computation (L=12, B=2, S=2048, D=1024, DK=256):
    normalized = LayerNorm_D(states)                    # (L,B,S,D)
    query  = normalized[-1] @ Wq.T                      # (B,S,DK)
    keys   = normalized @ Wk.T                          # (L,B,S,DK)
    logits = einsum('bsk,lbsk->lbs', query, keys)/16    # (L,B,S)
    w      = softmax_l(logits)
    mixed  = einsum('lbs,lbsd->bsd', w, states)
    out    = g*states[-1] + (1-g)*mixed,  g = sigmoid(latest_gate)

Algebra: logits[l,n] = (u[n].x[l,n] - mu[l,n]*C1[n] + C2[n]) * r[l,n] / 16
with u[n] = Wk.T(Wq norm11[n]) (uw = u*ln_w folded), C1 = sum(uw), C2 = u.ln_b,
mu/r the LN mean / rsqrt(var+eps) of layer l.  v2 additionally centers u:
    u' = uw - C1/D   =>   logits[l,n] = (u'[n].x[l,n] + C2[n]) * r[l,n] / 16
which removes the explicit mean correction from the logits path (the dot
against the centered u' absorbs it exactly).

v2 layout/engine plan (per 128-position tile, all-bf16 on-chip):
  - one SWDGE cast-DMA loads the tile's 12 layers f32->bf16 from a
    position-major [npc, L, D] DRAM shard (contiguous 48KB per partition)
  - per-layer var: layers in BNS_LAYERS use DVE bn_stats; the rest compute
    sum(x) on GPSIMD tensor_reduce and sum(x^2) on ACT Square+accum
  - dots A'[l] = u'.x[l]: DVE tensor_tensor mult (bf16 2x) + tensor_scalar
    reduce (bf16 4x) -- the fused scalar_tensor_tensor is 1x-only, slower
  - softmax over l with gate folded into the weights
  - mixed: PSUM-accumulated diag(w_l) matmuls on TensorE, bf16
Sharding: positions (b*S+s) split contiguously across 8 cores; pointwise in
position, no collectives.
"""

import math
from contextlib import ExitStack

import numpy as np

import concourse.bacc as bacc
import concourse.mybir as mybir
import concourse.tile as tile
from concourse import masks
from concourse.bass_utils import run_bass_kernel_spmd

L, B, S, D, DK = 12, 2, 2048, 1024, 256
N_CORES = 8
NTOT = B * S            # 4096 positions
NPC = NTOT // N_CORES   # 512 positions per core
P = 128                 # SBUF partitions
LN_EPS = 1e-5
SCALE = 1.0 / math.sqrt(DK)

F32 = mybir.dt.float32
BF16 = mybir.dt.bfloat16
U32 = mybir.dt.uint32
ALU = mybir.AluOpType
ACTF = mybir.ActivationFunctionType

RSQRT_MAGIC = 0x5F3759DF

# dot-product implementation: "stt" = fused scalar_tensor_tensor,
# "tt_ts" = tensor_tensor mult + tensor_scalar accum-reduce,
# "amr" = affine_mul_reduce
DOT_MODE = "amr"
# how many of the 12 dot reductions run on ACT Copy+accum (only for tt_ts)
N_DOT_RED_ACT = 0
# input load path: "cast" = one SWDGE f32->bf16 cast DMA (299 GB/s),
# "f32act" = HWDGE f32 load (443 GB/s) + chunked ACT Copy converts to bf16
LOAD_MODE = "cast"
# ACT convert chunk size in layers (f32act mode)
ACT_CONV_CHUNK = 4


def _rsqrt_newton(nc, pool, vpe, r_out, ncols, n_iter=2):
    """r_out = rsqrt(vpe) via bit-trick seed + Newton iterations (pure DVE)."""
    magic = pool.tile([P, ncols], U32, tag="rs_magic")
    nc.vector.memset(magic[:], RSQRT_MAGIC)
    shifted = pool.tile([P, ncols], U32, tag="rs_shift")
    nc.vector.tensor_scalar(
        out=shifted[:], in0=vpe[:].bitcast(U32), scalar1=1, scalar2=None,
        op0=ALU.logical_shift_right,
    )
    yu = pool.tile([P, ncols], U32, tag="rs_seed")
    nc.vector.tensor_tensor(out=yu[:], in0=magic[:], in1=shifted[:], op=ALU.subtract)
    y = yu[:].bitcast(F32)
    t = pool.tile([P, ncols], F32, tag="rs_tmp")
    for _ in range(n_iter):
        # y <- y * (1.5 - 0.5 * vpe * y^2)
        nc.vector.tensor_tensor(out=t[:], in0=y, in1=y, op=ALU.mult)
        nc.vector.tensor_tensor(out=t[:], in0=t[:], in1=vpe[:], op=ALU.mult)
        nc.vector.tensor_scalar(
            out=t[:], in0=t[:], scalar1=-0.5, scalar2=1.5, op0=ALU.mult, op1=ALU.add,
        )
        nc.vector.tensor_tensor(out=t[:], in0=y, in1=t[:], op=ALU.mult)
        nc.vector.tensor_copy(r_out[:], t[:])
    return r_out


def build_program(npc, gate, use_affine, bench_loop=0):
    """Build the per-core SPMD Bass program.

    npc: positions handled by this core (multiple of 128).
    gate: float python scalar sigmoid(latest_gate), baked as immediates.
    use_affine: apply general ln_weight/ln_bias path (False when w==1,b==0).
    bench_loop: if > 0, wrap the whole body in a hardware loop repeating it
        bench_loop times (timing only).
    """
    assert npc % P == 0
    nt = npc // P
    g = float(gate)

    nc = bacc.Bacc("TRN2", target_bir_lowering=False, debug=False)

    # position-major shard: [npc, L, D]
    x_dram = nc.dram_tensor("states_shard", [npc, L, D], F32, kind="ExternalInput")
    # wqt: [128, 8*256] bf16; chunk c cols hold Wq.T[c*128:(c+1)*128, :]
    wqt_dram = nc.dram_tensor("wqt", [P, 8 * DK], BF16, kind="ExternalInput")
    # wk: [128, 2*1024] bf16; chunk h cols hold Wk[h*128:(h+1)*128, :]
    wk_dram = nc.dram_tensor("wk", [P, 2 * D], BF16, kind="ExternalInput")
    if use_affine:
        lnw_dram = nc.dram_tensor("lnw", [1, D], F32, kind="ExternalInput")
        lnb_dram = nc.dram_tensor("lnb", [1, D], F32, kind="ExternalInput")
    out_dram = nc.dram_tensor("out", [npc, D], F32, kind="ExternalOutput")

    with tile.TileContext(nc) as tc, ExitStack() as ctx:
        cpool = ctx.enter_context(tc.tile_pool(name="consts", bufs=1))
        xpool = ctx.enter_context(tc.tile_pool(name="x", bufs=3))
        if LOAD_MODE != "cast":
            xfpool = ctx.enter_context(tc.tile_pool(name="xf", bufs=2))
        spool = ctx.enter_context(tc.tile_pool(name="stats", bufs=2))
        dpool = ctx.enter_context(tc.tile_pool(name="dump", bufs=4))
        npool = ctx.enter_context(tc.tile_pool(name="n11", bufs=2))
        opool = ctx.enter_context(tc.tile_pool(name="osb", bufs=2))
        dgpool = ctx.enter_context(tc.tile_pool(name="dg", bufs=4))
        pT = ctx.enter_context(tc.tile_pool(name="psum_T", bufs=1, space="PSUM"))
        pQ = ctx.enter_context(tc.tile_pool(name="psum_q", bufs=1, space="PSUM"))
        pU = ctx.enter_context(tc.tile_pool(name="psum_u", bufs=1, space="PSUM"))
        pM = ctx.enter_context(tc.tile_pool(name="psum_m", bufs=1, space="PSUM"))

        # ---- constants ----
        ident_f = cpool.tile([P, P], F32)
        masks.make_identity(nc, ident_f[:])
        ident = cpool.tile([P, P], BF16)
        nc.scalar.copy(ident[:], ident_f[:])
        wqt = cpool.tile([P, 8 * DK], BF16)
        nc.sync.dma_start(wqt[:], wqt_dram[:])
        wk = cpool.tile([P, 2 * D], BF16)
        nc.sync.dma_start(wk[:], wk_dram[:])
        if use_affine:
            lnw_bc = cpool.tile([P, D], F32)
            nc.sync.dma_start(lnw_bc[0:1, :], lnw_dram[:])
            nc.gpsimd.partition_broadcast(lnw_bc[:], lnw_bc[0:1, :])
            lnb_bc = cpool.tile([P, D], F32)
            nc.sync.dma_start(lnb_bc[0:1, :], lnb_dram[:])
            nc.gpsimd.partition_broadcast(lnb_bc[:], lnb_bc[0:1, :])

        loop_ctx = tc.For_i(0, bench_loop, 1) if bench_loop > 0 else None
        if loop_ctx is not None:
            ctx.enter_context(loop_ctx)

        for t in range(nt):
            r0 = t * P
            xt = xpool.tile([P, L, D], BF16, tag="xt")
            if LOAD_MODE == "cast":
                # one SWDGE cast DMA, f32 -> bf16, 6 MB
                nc.gpsimd.dma_start(xt[:], x_dram[r0:r0 + P, :, :])
            else:
                # HWDGE f32 load + chunked ACT converts
                xf = xfpool.tile([P, L, D], F32, tag="xf")
                cs = ACT_CONV_CHUNK
                for c0 in range(0, L, cs):
                    nc.sync.dma_start(xf[:, c0:c0 + cs, :],
                                      x_dram[r0:r0 + P, c0:c0 + cs, :])
                    nc.scalar.activation(out=xt[:, c0:c0 + cs, :],
                                         in_=xf[:, c0:c0 + cs, :],
                                         func=ACTF.Copy)

            st = spool.tile([P, L, 12], F32, tag="st")
            ag = spool.tile([P, L, 2], F32, tag="ag")
            acol = spool.tile([P, L], F32, tag="acol")

            # ---- phase A: layer-11 stats -> n11 -> q -> u -> u' ----
            with tc.high_priority():
                nc.vector.bn_stats(st[:, L - 1, 0:6], xt[:, L - 1, 0:512])
                nc.vector.bn_stats(st[:, L - 1, 6:12], xt[:, L - 1, 512:1024])
                nc.vector.bn_aggr(ag[:, L - 1, :], st[:, L - 1, :])
                vpe11 = spool.tile([P, 1], F32, tag="vpe11")
                nc.vector.tensor_scalar(out=vpe11[:], in0=ag[:, L - 1, 1:2],
                                        scalar1=LN_EPS, scalar2=None, op0=ALU.add)
                r11 = spool.tile([P, 1], F32, tag="r11")
                _rsqrt_newton(nc, spool, vpe11, r11, 1)
                negmur = spool.tile([P, 1], F32, tag="negmur")
                nc.vector.tensor_tensor(out=negmur[:], in0=ag[:, L - 1, 0:1],
                                        in1=r11[:], op=ALU.mult)
                nc.vector.tensor_scalar(out=negmur[:], in0=negmur[:], scalar1=-1.0,
                                        scalar2=None, op0=ALU.mult)
                n11 = npool.tile([P, D], BF16, tag="n11")
                nc.vector.tensor_scalar(
                    out=n11[:], in0=xt[:, L - 1, :], scalar1=r11[:],
                    scalar2=negmur[:], op0=ALU.mult, op1=ALU.add,
                )
                if use_affine:
                    nc.vector.tensor_tensor(out=n11[:], in0=n11[:], in1=lnw_bc[:],
                                            op=ALU.mult)
                    nc.vector.tensor_tensor(out=n11[:], in0=n11[:], in1=lnb_bc[:],
                                            op=ALU.add)
                # transpose n11 (8x 128x128 on TensorE), copy to SBUF bf16
                pt = pT.tile([P, D], BF16, tag="pT")
                for c in range(8):
                    nc.tensor.transpose(pt[:, c * P:(c + 1) * P],
                                        n11[:, c * P:(c + 1) * P], ident[:])
                n11t = npool.tile([P, D], BF16, tag="n11t")
                nc.scalar.copy(n11t[:], pt[:])
                # q^T halves: [dk-half 128, pos 128]
                qs = npool.tile([P, 2 * P], BF16, tag="qs")
                for h in range(2):
                    pq = pQ.tile([P, P], F32, tag="pq")
                    for c in range(8):
                        nc.tensor.matmul(
                            pq[:],
                            lhsT=wqt[:, c * DK + h * P: c * DK + (h + 1) * P],
                            rhs=n11t[:, c * P:(c + 1) * P],
                            start=(c == 0), stop=(c == 7),
                        )
                    nc.scalar.copy(qs[:, h * P:(h + 1) * P], pq[:])
                # u[pos, d] = Wk.T q
                pu = pU.tile([P, D], F32, tag="pu")
                for h in range(2):
                    for nh in range(2):
                        nc.tensor.matmul(
                            pu[:, nh * 512:(nh + 1) * 512],
                            lhsT=qs[:, h * P:(h + 1) * P],
                            rhs=wk[:, h * D + nh * 512: h * D + (nh + 1) * 512],
                            start=(h == 0), stop=(h == 1),
                        )
                usb = npool.tile([P, D], BF16, tag="usb")
                c1 = spool.tile([P, 1], F32, tag="c1")
                nc.scalar.activation(out=usb[:], in_=pu[:], func=ACTF.Copy,
                                     accum_out=(None if use_affine else c1[:]))
                if use_affine:
                    # uw = u * ln_w ; C2 = u . ln_b ; C1 = sum(uw)
                    c2 = spool.tile([P, 1], F32, tag="c2")
                    prb = dpool.tile([P, D], F32, tag="c2p")
                    nc.vector.scalar_tensor_tensor(
                        out=prb[:], in0=usb[:], scalar=0.0, in1=lnb_bc[:],
                        op0=ALU.add, op1=ALU.mult, accum_out=c2[:])
                    nc.vector.tensor_tensor(out=usb[:], in0=usb[:], in1=lnw_bc[:],
                                            op=ALU.mult)
                    nc.vector.tensor_reduce(out=c1[:], in_=usb[:],
                                            axis=mybir.AxisListType.X, op=ALU.add)
                # u' = u - C1/D  (centering absorbs the mean correction)
                negc1d = spool.tile([P, 1], F32, tag="negc1d")
                nc.vector.tensor_scalar(out=negc1d[:], in0=c1[:],
                                        scalar1=-1.0 / D, scalar2=None,
                                        op0=ALU.mult)
                up = npool.tile([P, D], BF16, tag="up")
                nc.vector.tensor_scalar(out=up[:], in0=usb[:], scalar1=1.0,
                                        scalar2=negc1d[:], op0=ALU.mult,
                                        op1=ALU.add)

            # ---- per-layer stats for l=0..10: DVE bn_stats (2x on bf16) ----
            for l in range(L - 1):
                nc.vector.bn_stats(st[:, l, 0:6], xt[:, l, 0:512])
                nc.vector.bn_stats(st[:, l, 6:12], xt[:, l, 512:1024])
                nc.vector.bn_aggr(ag[:, l, :], st[:, l, :])

            # ---- dots: A'[l] = u' . x_l ----
            for l in range(L):
                if DOT_MODE == "stt":
                    dmp = dpool.tile([P, D], BF16, tag="dmp")
                    nc.vector.scalar_tensor_tensor(
                        out=dmp[:], in0=xt[:, l, :], scalar=0.0, in1=up[:],
                        op0=ALU.add, op1=ALU.mult,
                        accum_out=acol[:, l:l + 1])
                elif DOT_MODE == "amr":
                    dmp = dpool.tile([P, D], BF16, tag="dmp")
                    nc.vector.affine_mul_reduce(
                        out=dmp[:], accum_out=acol[:, l:l + 1],
                        in0=xt[:, l, :], in1=up[:], scale=1.0, bias=0.0)
                else:
                    pr = dpool.tile([P, D], BF16, tag="pr")
                    nc.vector.tensor_tensor(out=pr[:], in0=xt[:, l, :], in1=up[:],
                                            op=ALU.mult)
                    if l < N_DOT_RED_ACT:
                        nc.scalar.activation(
                            out=dpool.tile([P, D], BF16, tag="dr"),
                            in_=pr[:], func=ACTF.Copy,
                            accum_out=acol[:, l:l + 1])
                    else:
                        dmp2 = dpool.tile([P, D], BF16, tag="dmp2")
                        nc.vector.tensor_scalar(out=dmp2[:], in0=pr[:],
                                                scalar1=1.0, scalar2=0.0,
                                                op0=ALU.mult, op1=ALU.add,
                                                accum_out=acol[:, l:l + 1])

            # ---- logits + softmax + gate fold ----
            vpe = spool.tile([P, L], F32, tag="vpe")
            nc.vector.tensor_scalar(out=vpe[:], in0=ag[:, :, 1],
                                    scalar1=LN_EPS, scalar2=None, op0=ALU.add)
            rr = spool.tile([P, L], F32, tag="rr")
            _rsqrt_newton(nc, spool, vpe, rr, L)
            lg = spool.tile([P, L], F32, tag="lg")
            nc.vector.tensor_tensor(out=lg[:], in0=acol[:], in1=rr[:],
                                    op=ALU.mult)
            if use_affine:
                mur = spool.tile([P, L], F32, tag="mur")
                nc.vector.tensor_scalar(out=mur[:], in0=rr[:],
                                        scalar1=c2[:], scalar2=None,
                                        op0=ALU.mult)
                nc.vector.tensor_tensor(out=lg[:], in0=lg[:], in1=mur[:],
                                        op=ALU.add)
            negmax = spool.tile([P, 1], F32, tag="negmax")
            nc.vector.tensor_reduce(out=negmax[:], in_=lg[:],
                                    axis=mybir.AxisListType.X, op=ALU.max,
                                    negate=True)
            nc.vector.tensor_scalar(out=negmax[:], in0=negmax[:], scalar1=SCALE,
                                    scalar2=None, op0=ALU.mult)
            wts = spool.tile([P, L], F32, tag="wts")
            ssum = spool.tile([P, 1], F32, tag="ssum")
            nc.scalar.activation(
                out=wts[:], in_=lg[:], func=ACTF.Exp, bias=negmax[:], scale=SCALE,
                accum_out=ssum[:],
            )
            rs = spool.tile([P, 1], F32, tag="rs")
            nc.vector.reciprocal(rs[:], ssum[:])
            nc.vector.tensor_scalar(out=rs[:], in0=rs[:], scalar1=(1.0 - g),
                                    scalar2=None, op0=ALU.mult)
            nc.vector.tensor_scalar(out=wts[:], in0=wts[:], scalar1=rs[:],
                                    scalar2=None, op0=ALU.mult)
            nc.vector.tensor_scalar(out=wts[:, L - 1:L], in0=wts[:, L - 1:L],
                                    scalar1=g, scalar2=None, op0=ALU.add)

            # ---- mixed: PSUM-accumulated diag matmuls (bf16) ----
            pm = pM.tile([P, D], F32, tag="pm")
            for l in range(L):
                dg = dgpool.tile([P, P], BF16, tag="dg")
                nc.vector.tensor_scalar(out=dg[:], in0=ident[:],
                                        scalar1=wts[:, l:l + 1], scalar2=None,
                                        op0=ALU.mult)
                for nh in range(2):
                    nc.tensor.matmul(
                        pm[:, nh * 512:(nh + 1) * 512],
                        lhsT=dg[:],
                        rhs=xt[:, l, nh * 512:(nh + 1) * 512],
                        start=(l == 0), stop=(l == L - 1),
                    )
            osb = opool.tile([P, D], F32, tag="osb")
            nc.scalar.copy(osb[:], pm[:])
            nc.sync.dma_start(out_dram[r0:r0 + P, :], osb[:])

    nc.compile()
    return nc


_PROGRAM_CACHE = {}


def _get_program(npc, gate, use_affine):
    key = (npc, round(float(gate), 10), bool(use_affine))
    if key not in _PROGRAM_CACHE:
        _PROGRAM_CACHE[key] = build_program(npc, gate, use_affine)
    return _PROGRAM_CACHE[key]


def prep_weights(Wq, Wk):
    """Host-side prep of the replicated small params (bf16 chunk layouts)."""
    bf = mybir.dt.np(BF16)
    wqt = np.ascontiguousarray(
        Wq.T.reshape(8, P, DK).transpose(1, 0, 2).reshape(P, 8 * DK)).astype(bf)
    wkr = np.ascontiguousarray(
        Wk.reshape(2, P, D).transpose(1, 0, 2).reshape(P, 2 * D)).astype(bf)
    return wqt, wkr


def prep_states(states):
    """[L,B,S,D] f32 -> position-major [NTOT, L, D] contiguous."""
    xs = np.asarray(states, dtype=np.float32).reshape(L, NTOT, D)
    return np.ascontiguousarray(xs.transpose(1, 0, 2))


def kernel(states, Wq, Wk, ln_weight, ln_bias, latest_gate, **_unused):
    Wq = np.asarray(Wq, dtype=np.float32)
    Wk = np.asarray(Wk, dtype=np.float32)
    ln_weight = np.asarray(ln_weight, dtype=np.float32)
    ln_bias = np.asarray(ln_bias, dtype=np.float32)
    gate = 1.0 / (1.0 + math.exp(-float(np.asarray(latest_gate))))

    use_affine = not (np.all(ln_weight == 1.0) and np.all(ln_bias == 0.0))
    nc = _get_program(NPC, gate, use_affine)

    wqt, wkr = prep_weights(Wq, Wk)
    xp = prep_states(states)

    in_maps = []
    for c in range(N_CORES):
        m = {
            "states_shard": np.ascontiguousarray(xp[c * NPC:(c + 1) * NPC]),
            "wqt": wqt,
            "wk": wkr,
        }
        if use_affine:
            m["lnw"] = ln_weight.reshape(1, D)
            m["lnb"] = ln_bias.reshape(1, D)
        in_maps.append(m)

    res = run_bass_kernel_spmd(nc, in_maps, list(range(N_CORES)))
    out = np.concatenate([res.results[c]["out"] for c in range(N_CORES)], axis=0)
    return np.ascontiguousarray(out.reshape(B, S, D).astype(np.float32))


# revision 8
# speedup vs baseline: 1.0782x; 1.0782x over previous
"""Trainium2 Bass kernel for CrossDepthAttentionResidual (v2, bf16 pipeline).

Reference computation (L=12, B=2, S=2048, D=1024, DK=256):
    normalized = LayerNorm_D(states)                    # (L,B,S,D)
    query  = normalized[-1] @ Wq.T                      # (B,S,DK)
    keys   = normalized @ Wk.T                          # (L,B,S,DK)
    logits = einsum('bsk,lbsk->lbs', query, keys)/16    # (L,B,S)
    w      = softmax_l(logits)
    mixed  = einsum('lbs,lbsd->bsd', w, states)
    out    = g*states[-1] + (1-g)*mixed,  g = sigmoid(latest_gate)

Algebra: logits[l,n] = (u[n].x[l,n] - mu[l,n]*C1[n] + C2[n]) * r[l,n] / 16
with u[n] = Wk.T(Wq norm11[n]) (uw = u*ln_w folded), C1 = sum(uw), C2 = u.ln_b,
mu/r the LN mean / rsqrt(var+eps) of layer l.  v2 additionally centers u:
    u' = uw - C1/D   =>   logits[l,n] = (u'[n].x[l,n] + C2[n]) * r[l,n] / 16
which removes the explicit mean correction from the logits path (the dot
against the centered u' absorbs it exactly).

v2 layout/engine plan (per 128-position tile, all-bf16 on-chip):
  - one SWDGE cast-DMA loads the tile's 12 layers f32->bf16 from a
    position-major [npc, L, D] DRAM shard (contiguous 48KB per partition)
  - per-layer var: layers in BNS_LAYERS use DVE bn_stats; the rest compute
    sum(x) on GPSIMD tensor_reduce and sum(x^2) on ACT Square+accum
  - dots A'[l] = u'.x[l]: DVE tensor_tensor mult (bf16 2x) + tensor_scalar
    reduce (bf16 4x) -- the fused scalar_tensor_tensor is 1x-only, slower
  - softmax over l with gate folded into the weights
  - mixed: PSUM-accumulated diag(w_l) matmuls on TensorE, bf16
Sharding: positions (b*S+s) split contiguously across 8 cores; pointwise in
position, no collectives.
"""

import math
from contextlib import ExitStack

import numpy as np

import concourse.bacc as bacc
import concourse.mybir as mybir
import concourse.tile as tile
from concourse import masks
from concourse.bass_utils import run_bass_kernel_spmd

L, B, S, D, DK = 12, 2, 2048, 1024, 256
N_CORES = 8
NTOT = B * S            # 4096 positions
NPC = NTOT // N_CORES   # 512 positions per core
P = 128                 # SBUF partitions
LN_EPS = 1e-5
SCALE = 1.0 / math.sqrt(DK)

F32 = mybir.dt.float32
BF16 = mybir.dt.bfloat16
U32 = mybir.dt.uint32
ALU = mybir.AluOpType
ACTF = mybir.ActivationFunctionType

RSQRT_MAGIC = 0x5F3759DF

# dot-product implementation: "stt" = fused scalar_tensor_tensor,
# "tt_ts" = tensor_tensor mult + tensor_scalar accum-reduce,
# "amr" = affine_mul_reduce
DOT_MODE = "amr"
# how many of the 12 dot reductions run on ACT Copy+accum (only for tt_ts)
N_DOT_RED_ACT = 0
# input load path: "cast" = one SWDGE f32->bf16 cast DMA (299 GB/s),
# "f32act" = HWDGE f32 load (443 GB/s) + chunked ACT Copy converts to bf16
LOAD_MODE = "f32act"
# ACT convert chunk size in layers (f32act mode)
ACT_CONV_CHUNK = 4


def _rsqrt_newton(nc, pool, vpe, r_out, ncols, n_iter=2):
    """r_out = rsqrt(vpe) via bit-trick seed + Newton iterations (pure DVE)."""
    magic = pool.tile([P, ncols], U32, tag="rs_magic")
    nc.vector.memset(magic[:], RSQRT_MAGIC)
    shifted = pool.tile([P, ncols], U32, tag="rs_shift")
    nc.vector.tensor_scalar(
        out=shifted[:], in0=vpe[:].bitcast(U32), scalar1=1, scalar2=None,
        op0=ALU.logical_shift_right,
    )
    yu = pool.tile([P, ncols], U32, tag="rs_seed")
    nc.vector.tensor_tensor(out=yu[:], in0=magic[:], in1=shifted[:], op=ALU.subtract)
    y = yu[:].bitcast(F32)
    t = pool.tile([P, ncols], F32, tag="rs_tmp")
    for _ in range(n_iter):
        # y <- y * (1.5 - 0.5 * vpe * y^2)
        nc.vector.tensor_tensor(out=t[:], in0=y, in1=y, op=ALU.mult)
        nc.vector.tensor_tensor(out=t[:], in0=t[:], in1=vpe[:], op=ALU.mult)
        nc.vector.tensor_scalar(
            out=t[:], in0=t[:], scalar1=-0.5, scalar2=1.5, op0=ALU.mult, op1=ALU.add,
        )
        nc.vector.tensor_tensor(out=t[:], in0=y, in1=t[:], op=ALU.mult)
        nc.vector.tensor_copy(r_out[:], t[:])
    return r_out


def build_program(npc, gate, use_affine, bench_loop=0):
    """Build the per-core SPMD Bass program.

    npc: positions handled by this core (multiple of 128).
    gate: float python scalar sigmoid(latest_gate), baked as immediates.
    use_affine: apply general ln_weight/ln_bias path (False when w==1,b==0).
    bench_loop: if > 0, wrap the whole body in a hardware loop repeating it
        bench_loop times (timing only).
    """
    assert npc % P == 0
    nt = npc // P
    g = float(gate)

    nc = bacc.Bacc("TRN2", target_bir_lowering=False, debug=False)

    # position-major shard: [npc, L, D]
    x_dram = nc.dram_tensor("states_shard", [npc, L, D], F32, kind="ExternalInput")
    # wqt: [128, 8*256] bf16; chunk c cols hold Wq.T[c*128:(c+1)*128, :]
    wqt_dram = nc.dram_tensor("wqt", [P, 8 * DK], BF16, kind="ExternalInput")
    # wk: [128, 2*1024] bf16; chunk h cols hold Wk[h*128:(h+1)*128, :]
    wk_dram = nc.dram_tensor("wk", [P, 2 * D], BF16, kind="ExternalInput")
    if use_affine:
        lnw_dram = nc.dram_tensor("lnw", [1, D], F32, kind="ExternalInput")
        lnb_dram = nc.dram_tensor("lnb", [1, D], F32, kind="ExternalInput")
    out_dram = nc.dram_tensor("out", [npc, D], F32, kind="ExternalOutput")

    with tile.TileContext(nc) as tc, ExitStack() as ctx:
        cpool = ctx.enter_context(tc.tile_pool(name="consts", bufs=1))
        xpool = ctx.enter_context(tc.tile_pool(name="x", bufs=(3 if LOAD_MODE == "cast" else 2)))
        if LOAD_MODE != "cast":
            xfpool = ctx.enter_context(tc.tile_pool(name="xf", bufs=2))
        spool = ctx.enter_context(tc.tile_pool(name="stats", bufs=2))
        dpool = ctx.enter_context(tc.tile_pool(name="dump", bufs=(4 if LOAD_MODE == "cast" else 2)))
        npool = ctx.enter_context(tc.tile_pool(name="n11", bufs=2))
        opool = ctx.enter_context(tc.tile_pool(name="osb", bufs=2))
        dgpool = ctx.enter_context(tc.tile_pool(name="dg", bufs=4))
        pT = ctx.enter_context(tc.tile_pool(name="psum_T", bufs=1, space="PSUM"))
        pQ = ctx.enter_context(tc.tile_pool(name="psum_q", bufs=1, space="PSUM"))
        pU = ctx.enter_context(tc.tile_pool(name="psum_u", bufs=1, space="PSUM"))
        pM = ctx.enter_context(tc.tile_pool(name="psum_m", bufs=1, space="PSUM"))

        # ---- constants ----
        ident_f = cpool.tile([P, P], F32)
        masks.make_identity(nc, ident_f[:])
        ident = cpool.tile([P, P], BF16)
        nc.scalar.copy(ident[:], ident_f[:])
        wqt = cpool.tile([P, 8 * DK], BF16)
        nc.sync.dma_start(wqt[:], wqt_dram[:])
        wk = cpool.tile([P, 2 * D], BF16)
        nc.sync.dma_start(wk[:], wk_dram[:])
        if use_affine:
            lnw_bc = cpool.tile([P, D], F32)
            nc.sync.dma_start(lnw_bc[0:1, :], lnw_dram[:])
            nc.gpsimd.partition_broadcast(lnw_bc[:], lnw_bc[0:1, :])
            lnb_bc = cpool.tile([P, D], F32)
            nc.sync.dma_start(lnb_bc[0:1, :], lnb_dram[:])
            nc.gpsimd.partition_broadcast(lnb_bc[:], lnb_bc[0:1, :])

        loop_ctx = tc.For_i(0, bench_loop, 1) if bench_loop > 0 else None
        if loop_ctx is not None:
            ctx.enter_context(loop_ctx)

        for t in range(nt):
            r0 = t * P
            xt = xpool.tile([P, L, D], BF16, tag="xt")
            if LOAD_MODE == "cast":
                # one SWDGE cast DMA, f32 -> bf16, 6 MB
                nc.gpsimd.dma_start(xt[:], x_dram[r0:r0 + P, :, :])
            else:
                # HWDGE f32 load + chunked ACT converts
                xf = xfpool.tile([P, L, D], F32, tag="xf")
                cs = ACT_CONV_CHUNK
                for c0 in range(0, L, cs):
                    nc.sync.dma_start(xf[:, c0:c0 + cs, :],
                                      x_dram[r0:r0 + P, c0:c0 + cs, :])
                    nc.scalar.activation(out=xt[:, c0:c0 + cs, :],
                                         in_=xf[:, c0:c0 + cs, :],
                                         func=ACTF.Copy)

            st = spool.tile([P, L, 12], F32, tag="st")
            ag = spool.tile([P, L, 2], F32, tag="ag")
            acol = spool.tile([P, L], F32, tag="acol")

            # ---- phase A: layer-11 stats -> n11 -> q -> u -> u' ----
            with tc.high_priority():
                nc.vector.bn_stats(st[:, L - 1, 0:6], xt[:, L - 1, 0:512])
                nc.vector.bn_stats(st[:, L - 1, 6:12], xt[:, L - 1, 512:1024])
                nc.vector.bn_aggr(ag[:, L - 1, :], st[:, L - 1, :])
                vpe11 = spool.tile([P, 1], F32, tag="vpe11")
                nc.vector.tensor_scalar(out=vpe11[:], in0=ag[:, L - 1, 1:2],
                                        scalar1=LN_EPS, scalar2=None, op0=ALU.add)
                r11 = spool.tile([P, 1], F32, tag="r11")
                _rsqrt_newton(nc, spool, vpe11, r11, 1)
                negmur = spool.tile([P, 1], F32, tag="negmur")
                nc.vector.tensor_tensor(out=negmur[:], in0=ag[:, L - 1, 0:1],
                                        in1=r11[:], op=ALU.mult)
                nc.vector.tensor_scalar(out=negmur[:], in0=negmur[:], scalar1=-1.0,
                                        scalar2=None, op0=ALU.mult)
                n11 = npool.tile([P, D], BF16, tag="n11")
                nc.vector.tensor_scalar(
                    out=n11[:], in0=xt[:, L - 1, :], scalar1=r11[:],
                    scalar2=negmur[:], op0=ALU.mult, op1=ALU.add,
                )
                if use_affine:
                    nc.vector.tensor_tensor(out=n11[:], in0=n11[:], in1=lnw_bc[:],
                                            op=ALU.mult)
                    nc.vector.tensor_tensor(out=n11[:], in0=n11[:], in1=lnb_bc[:],
                                            op=ALU.add)
                # transpose n11 (8x 128x128 on TensorE), copy to SBUF bf16
                pt = pT.tile([P, D], BF16, tag="pT")
                for c in range(8):
                    nc.tensor.transpose(pt[:, c * P:(c + 1) * P],
                                        n11[:, c * P:(c + 1) * P], ident[:])
                n11t = npool.tile([P, D], BF16, tag="n11t")
                nc.scalar.copy(n11t[:], pt[:])
                # q^T halves: [dk-half 128, pos 128]
                qs = npool.tile([P, 2 * P], BF16, tag="qs")
                for h in range(2):
                    pq = pQ.tile([P, P], F32, tag="pq")
                    for c in range(8):
                        nc.tensor.matmul(
                            pq[:],
                            lhsT=wqt[:, c * DK + h * P: c * DK + (h + 1) * P],
                            rhs=n11t[:, c * P:(c + 1) * P],
                            start=(c == 0), stop=(c == 7),
                        )
                    nc.scalar.copy(qs[:, h * P:(h + 1) * P], pq[:])
                # u[pos, d] = Wk.T q
                pu = pU.tile([P, D], F32, tag="pu")
                for h in range(2):
                    for nh in range(2):
                        nc.tensor.matmul(
                            pu[:, nh * 512:(nh + 1) * 512],
                            lhsT=qs[:, h * P:(h + 1) * P],
                            rhs=wk[:, h * D + nh * 512: h * D + (nh + 1) * 512],
                            start=(h == 0), stop=(h == 1),
                        )
                usb = npool.tile([P, D], BF16, tag="usb")
                c1 = spool.tile([P, 1], F32, tag="c1")
                nc.scalar.activation(out=usb[:], in_=pu[:], func=ACTF.Copy,
                                     accum_out=(None if use_affine else c1[:]))
                if use_affine:
                    # uw = u * ln_w ; C2 = u . ln_b ; C1 = sum(uw)
                    c2 = spool.tile([P, 1], F32, tag="c2")
                    prb = dpool.tile([P, D], F32, tag="c2p")
                    nc.vector.scalar_tensor_tensor(
                        out=prb[:], in0=usb[:], scalar=0.0, in1=lnb_bc[:],
                        op0=ALU.add, op1=ALU.mult, accum_out=c2[:])
                    nc.vector.tensor_tensor(out=usb[:], in0=usb[:], in1=lnw_bc[:],
                                            op=ALU.mult)
                    nc.vector.tensor_reduce(out=c1[:], in_=usb[:],
                                            axis=mybir.AxisListType.X, op=ALU.add)
                # u' = u - C1/D  (centering absorbs the mean correction)
                negc1d = spool.tile([P, 1], F32, tag="negc1d")
                nc.vector.tensor_scalar(out=negc1d[:], in0=c1[:],
                                        scalar1=-1.0 / D, scalar2=None,
                                        op0=ALU.mult)
                up = npool.tile([P, D], BF16, tag="up")
                nc.vector.tensor_scalar(out=up[:], in0=usb[:], scalar1=1.0,
                                        scalar2=negc1d[:], op0=ALU.mult,
                                        op1=ALU.add)

            # ---- per-layer stats for l=0..10: DVE bn_stats (2x on bf16) ----
            for l in range(L - 1):
                nc.vector.bn_stats(st[:, l, 0:6], xt[:, l, 0:512])
                nc.vector.bn_stats(st[:, l, 6:12], xt[:, l, 512:1024])
                nc.vector.bn_aggr(ag[:, l, :], st[:, l, :])

            # ---- dots: A'[l] = u' . x_l ----
            for l in range(L):
                if DOT_MODE == "stt":
                    dmp = dpool.tile([P, D], BF16, tag="dmp")
                    nc.vector.scalar_tensor_tensor(
                        out=dmp[:], in0=xt[:, l, :], scalar=0.0, in1=up[:],
                        op0=ALU.add, op1=ALU.mult,
                        accum_out=acol[:, l:l + 1])
                elif DOT_MODE == "amr":
                    dmp = dpool.tile([P, D], BF16, tag="dmp")
                    nc.vector.affine_mul_reduce(
                        out=dmp[:], accum_out=acol[:, l:l + 1],
                        in0=xt[:, l, :], in1=up[:], scale=1.0, bias=0.0)
                else:
                    pr = dpool.tile([P, D], BF16, tag="pr")
                    nc.vector.tensor_tensor(out=pr[:], in0=xt[:, l, :], in1=up[:],
                                            op=ALU.mult)
                    if l < N_DOT_RED_ACT:
                        nc.scalar.activation(
                            out=dpool.tile([P, D], BF16, tag="dr"),
                            in_=pr[:], func=ACTF.Copy,
                            accum_out=acol[:, l:l + 1])
                    else:
                        dmp2 = dpool.tile([P, D], BF16, tag="dmp2")
                        nc.vector.tensor_scalar(out=dmp2[:], in0=pr[:],
                                                scalar1=1.0, scalar2=0.0,
                                                op0=ALU.mult, op1=ALU.add,
                                                accum_out=acol[:, l:l + 1])

            # ---- logits + softmax + gate fold ----
            vpe = spool.tile([P, L], F32, tag="vpe")
            nc.vector.tensor_scalar(out=vpe[:], in0=ag[:, :, 1],
                                    scalar1=LN_EPS, scalar2=None, op0=ALU.add)
            rr = spool.tile([P, L], F32, tag="rr")
            _rsqrt_newton(nc, spool, vpe, rr, L)
            lg = spool.tile([P, L], F32, tag="lg")
            nc.vector.tensor_tensor(out=lg[:], in0=acol[:], in1=rr[:],
                                    op=ALU.mult)
            if use_affine:
                mur = spool.tile([P, L], F32, tag="mur")
                nc.vector.tensor_scalar(out=mur[:], in0=rr[:],
                                        scalar1=c2[:], scalar2=None,
                                        op0=ALU.mult)
                nc.vector.tensor_tensor(out=lg[:], in0=lg[:], in1=mur[:],
                                        op=ALU.add)
            negmax = spool.tile([P, 1], F32, tag="negmax")
            nc.vector.tensor_reduce(out=negmax[:], in_=lg[:],
                                    axis=mybir.AxisListType.X, op=ALU.max,
                                    negate=True)
            nc.vector.tensor_scalar(out=negmax[:], in0=negmax[:], scalar1=SCALE,
                                    scalar2=None, op0=ALU.mult)
            wts = spool.tile([P, L], F32, tag="wts")
            ssum = spool.tile([P, 1], F32, tag="ssum")
            nc.scalar.activation(
                out=wts[:], in_=lg[:], func=ACTF.Exp, bias=negmax[:], scale=SCALE,
                accum_out=ssum[:],
            )
            rs = spool.tile([P, 1], F32, tag="rs")
            nc.vector.reciprocal(rs[:], ssum[:])
            nc.vector.tensor_scalar(out=rs[:], in0=rs[:], scalar1=(1.0 - g),
                                    scalar2=None, op0=ALU.mult)
            nc.vector.tensor_scalar(out=wts[:], in0=wts[:], scalar1=rs[:],
                                    scalar2=None, op0=ALU.mult)
            nc.vector.tensor_scalar(out=wts[:, L - 1:L], in0=wts[:, L - 1:L],
                                    scalar1=g, scalar2=None, op0=ALU.add)

            # ---- mixed: PSUM-accumulated diag matmuls (bf16) ----
            pm = pM.tile([P, D], F32, tag="pm")
            for l in range(L):
                dg = dgpool.tile([P, P], BF16, tag="dg")
                nc.vector.tensor_scalar(out=dg[:], in0=ident[:],
                                        scalar1=wts[:, l:l + 1], scalar2=None,
                                        op0=ALU.mult)
                for nh in range(2):
                    nc.tensor.matmul(
                        pm[:, nh * 512:(nh + 1) * 512],
                        lhsT=dg[:],
                        rhs=xt[:, l, nh * 512:(nh + 1) * 512],
                        start=(l == 0), stop=(l == L - 1),
                    )
            osb = opool.tile([P, D], F32, tag="osb")
            nc.scalar.copy(osb[:], pm[:])
            nc.sync.dma_start(out_dram[r0:r0 + P, :], osb[:])

    nc.compile()
    return nc


_PROGRAM_CACHE = {}


def _get_program(npc, gate, use_affine):
    key = (npc, round(float(gate), 10), bool(use_affine))
    if key not in _PROGRAM_CACHE:
        _PROGRAM_CACHE[key] = build_program(npc, gate, use_affine)
    return _PROGRAM_CACHE[key]


def prep_weights(Wq, Wk):
    """Host-side prep of the replicated small params (bf16 chunk layouts)."""
    bf = mybir.dt.np(BF16)
    wqt = np.ascontiguousarray(
        Wq.T.reshape(8, P, DK).transpose(1, 0, 2).reshape(P, 8 * DK)).astype(bf)
    wkr = np.ascontiguousarray(
        Wk.reshape(2, P, D).transpose(1, 0, 2).reshape(P, 2 * D)).astype(bf)
    return wqt, wkr


def prep_states(states):
    """[L,B,S,D] f32 -> position-major [NTOT, L, D] contiguous."""
    xs = np.asarray(states, dtype=np.float32).reshape(L, NTOT, D)
    return np.ascontiguousarray(xs.transpose(1, 0, 2))


def kernel(states, Wq, Wk, ln_weight, ln_bias, latest_gate, **_unused):
    Wq = np.asarray(Wq, dtype=np.float32)
    Wk = np.asarray(Wk, dtype=np.float32)
    ln_weight = np.asarray(ln_weight, dtype=np.float32)
    ln_bias = np.asarray(ln_bias, dtype=np.float32)
    gate = 1.0 / (1.0 + math.exp(-float(np.asarray(latest_gate))))

    use_affine = not (np.all(ln_weight == 1.0) and np.all(ln_bias == 0.0))
    nc = _get_program(NPC, gate, use_affine)

    wqt, wkr = prep_weights(Wq, Wk)
    xp = prep_states(states)

    in_maps = []
    for c in range(N_CORES):
        m = {
            "states_shard": np.ascontiguousarray(xp[c * NPC:(c + 1) * NPC]),
            "wqt": wqt,
            "wk": wkr,
        }
        if use_affine:
            m["lnw"] = ln_weight.reshape(1, D)
            m["lnb"] = ln_bias.reshape(1, D)
        in_maps.append(m)

    res = run_bass_kernel_spmd(nc, in_maps, list(range(N_CORES)))
    out = np.concatenate([res.results[c]["out"] for c in range(N_CORES)], axis=0)
    return np.ascontiguousarray(out.reshape(B, S, D).astype(np.float32))


# revision 12
# speedup vs baseline: 1.1445x; 1.0615x over previous
"""Trainium2 Bass kernel for CrossDepthAttentionResidual (v3, bf16 pipeline,
stage-rotated software pipelining).

Reference computation (L=12, B=2, S=2048, D=1024, DK=256):
    normalized = LayerNorm_D(states)                    # (L,B,S,D)
    query  = normalized[-1] @ Wq.T                      # (B,S,DK)
    keys   = normalized @ Wk.T                          # (L,B,S,DK)
    logits = einsum('bsk,lbsk->lbs', query, keys)/16    # (L,B,S)
    w      = softmax_l(logits)
    mixed  = einsum('lbs,lbsd->bsd', w, states)
    out    = g*states[-1] + (1-g)*mixed,  g = sigmoid(latest_gate)

Algebra: logits[l,n] = (u[n].x[l,n] - mu[l,n]*C1[n] + C2[n]) * r[l,n] / 16
with u[n] = Wk.T(Wq norm11[n]) (uw = u*ln_w folded), C1 = sum(uw), C2 = u.ln_b,
mu/r the LN mean / rsqrt(var+eps) of layer l.  v3 centers u:
    u' = uw - C1/D   =>   logits[l,n] = (u'[n].x[l,n] + C2[n]) * r[l,n] / 16
so the centered dot absorbs the mean correction exactly.

Per-tile work (128 positions, all-bf16 on-chip):
  S0  load: HWDGE f32 chunk DMAs + ACT Copy converts to bf16 (the SWDGE
      f32->bf16 cast DMA path measures only ~300 GB/s vs ~443 plain)
  S1  phase A: layer-11 bn_stats -> n11 -> PE transposes -> q -> u -> u'
      centering, plus bn_stats for layers 0..10 (DVE 2x on bf16)
  S2  dots: A'[l] = u'.x_l via DVE affine_mul_reduce (fastest measured
      fused multiply-reduce; scalar_tensor_tensor runs 1x)
  S3  logits: vpe -> rsqrt (1 Newton step) -> lg -> negmax -> ACT exp
  S4  weights fold (gate into softmax weights), diag builds, PSUM-
      accumulated diag(w_l) matmuls on TensorE, out copy + store

Stages are emitted ROTATED across tiles (older tiles' later stages first)
so each engine's in-order instruction stream always has ready work queued
and every cross-engine round trip (ACT exp, PE u matmuls) lands a full
pipeline step before its consumer.

Sharding: positions (b*S+s) split contiguously across 8 cores; pointwise in
position, no collectives.
"""

import math
from contextlib import ExitStack

import numpy as np

import concourse.bacc as bacc
import concourse.mybir as mybir
import concourse.tile as tile
from concourse import masks
from concourse.bass_utils import run_bass_kernel_spmd

L, B, S, D, DK = 12, 2, 2048, 1024, 256
N_CORES = 8
NTOT = B * S            # 4096 positions
NPC = NTOT // N_CORES   # 512 positions per core
P = 128                 # SBUF partitions
LN_EPS = 1e-5
SCALE = 1.0 / math.sqrt(DK)

F32 = mybir.dt.float32
BF16 = mybir.dt.bfloat16
U32 = mybir.dt.uint32
ALU = mybir.AluOpType
ACTF = mybir.ActivationFunctionType

RSQRT_MAGIC = 0x5F3759DF

# ACT-convert chunk size in layers (S0)
CONV_CHUNK = 4


def _rsqrt_newton(nc, pool, vpe, r_out, ncols, n_iter=1):
    """r_out = rsqrt(vpe) via bit-trick seed + Newton iterations (pure DVE).

    One iteration leaves ~0.2% relative error -- far inside the softmax
    tolerance here (logits ~0.3 in magnitude).
    """
    magic = pool.tile([P, ncols], U32, tag="rs_magic")
    nc.vector.memset(magic[:], RSQRT_MAGIC)
    shifted = pool.tile([P, ncols], U32, tag="rs_shift")
    nc.vector.tensor_scalar(
        out=shifted[:], in0=vpe[:].bitcast(U32), scalar1=1, scalar2=None,
        op0=ALU.logical_shift_right,
    )
    yu = pool.tile([P, ncols], U32, tag="rs_seed")
    nc.vector.tensor_tensor(out=yu[:], in0=magic[:], in1=shifted[:], op=ALU.subtract)
    y = yu[:].bitcast(F32)
    t = pool.tile([P, ncols], F32, tag="rs_tmp")
    for i in range(n_iter):
        # y <- y * (1.5 - 0.5 * vpe * y^2)
        nc.vector.tensor_tensor(out=t[:], in0=y, in1=y, op=ALU.mult)
        nc.vector.tensor_tensor(out=t[:], in0=t[:], in1=vpe[:], op=ALU.mult)
        nc.vector.tensor_scalar(
            out=t[:], in0=t[:], scalar1=-0.5, scalar2=1.5, op0=ALU.mult, op1=ALU.add,
        )
        nc.vector.tensor_tensor(out=r_out[:], in0=y, in1=t[:], op=ALU.mult)
        y = r_out[:]
    return r_out


def build_program(npc, gate, use_affine, bench_loop=0):
    """Build the per-core SPMD Bass program.

    npc: positions handled by this core (multiple of 128).
    gate: float python scalar sigmoid(latest_gate), baked as immediates.
    use_affine: apply general ln_weight/ln_bias path (False when w==1,b==0).
    bench_loop: if > 0, wrap the whole body in a hardware loop repeating it
        bench_loop times (timing only).
    """
    assert npc % P == 0
    nt = npc // P
    g = float(gate)

    nc = bacc.Bacc("TRN2", target_bir_lowering=False, debug=False)

    # position-major shard: [npc, L, D]
    x_dram = nc.dram_tensor("states_shard", [npc, L, D], F32, kind="ExternalInput")
    # wqt: [128, 8*256] bf16; chunk c cols hold Wq.T[c*128:(c+1)*128, :]
    wqt_dram = nc.dram_tensor("wqt", [P, 8 * DK], BF16, kind="ExternalInput")
    # wk: [128, 2*1024] bf16; chunk h cols hold Wk[h*128:(h+1)*128, :]
    wk_dram = nc.dram_tensor("wk", [P, 2 * D], BF16, kind="ExternalInput")
    if use_affine:
        lnw_dram = nc.dram_tensor("lnw", [1, D], F32, kind="ExternalInput")
        lnb_dram = nc.dram_tensor("lnb", [1, D], F32, kind="ExternalInput")
    out_dram = nc.dram_tensor("out", [npc, D], F32, kind="ExternalOutput")

    with tile.TileContext(nc) as tc, ExitStack() as ctx:
        cpool = ctx.enter_context(tc.tile_pool(name="consts", bufs=1))
        xpool = ctx.enter_context(tc.tile_pool(name="x", bufs=4))
        xfpool = ctx.enter_context(tc.tile_pool(name="xf", bufs=3))
        spool = ctx.enter_context(tc.tile_pool(name="stats", bufs=4))
        dpool = ctx.enter_context(tc.tile_pool(name="dump", bufs=3))
        npool = ctx.enter_context(tc.tile_pool(name="n11", bufs=2))
        opool = ctx.enter_context(tc.tile_pool(name="osb", bufs=2))
        dgpool = ctx.enter_context(tc.tile_pool(name="dg", bufs=4))
        pTU = ctx.enter_context(tc.tile_pool(name="psum_tu", bufs=2, space="PSUM"))
        pQ = ctx.enter_context(tc.tile_pool(name="psum_q", bufs=1, space="PSUM"))
        pM = ctx.enter_context(tc.tile_pool(name="psum_m", bufs=1, space="PSUM"))

        # ---- constants ----
        ident_f = cpool.tile([P, P], F32)
        masks.make_identity(nc, ident_f[:])
        ident = cpool.tile([P, P], BF16)
        nc.scalar.copy(ident[:], ident_f[:])
        wqt = cpool.tile([P, 8 * DK], BF16)
        nc.sync.dma_start(wqt[:], wqt_dram[:])
        wk = cpool.tile([P, 2 * D], BF16)
        nc.sync.dma_start(wk[:], wk_dram[:])
        if use_affine:
            lnw_bc = cpool.tile([P, D], F32)
            nc.sync.dma_start(lnw_bc[0:1, :], lnw_dram[:])
            nc.gpsimd.partition_broadcast(lnw_bc[:], lnw_bc[0:1, :])
            lnb_bc = cpool.tile([P, D], F32)
            nc.sync.dma_start(lnb_bc[0:1, :], lnb_dram[:])
            nc.gpsimd.partition_broadcast(lnb_bc[:], lnb_bc[0:1, :])

        loop_ctx = tc.For_i(0, bench_loop, 1) if bench_loop > 0 else None
        if loop_ctx is not None:
            ctx.enter_context(loop_ctx)

        T = [dict() for _ in range(nt)]   # per-tile live tiles

        def s0_load(t):
            r0 = t * P
            xt = xpool.tile([P, L, D], BF16, tag="xt")
            for c0 in range(0, L, CONV_CHUNK):
                xf = xfpool.tile([P, CONV_CHUNK, D], F32, tag="xf")
                nc.sync.dma_start(xf[:], x_dram[r0:r0 + P, c0:c0 + CONV_CHUNK, :])
                nc.scalar.activation(out=xt[:, c0:c0 + CONV_CHUNK, :],
                                     in_=xf[:], func=ACTF.Copy)
            T[t]["xt"] = xt

        def s1_phase_a(t):
            xt = T[t]["xt"]
            st = spool.tile([P, L, 12], F32, tag="st")
            ag = spool.tile([P, L, 2], F32, tag="ag")
            T[t]["st"], T[t]["ag"] = st, ag
            # layer-11 stats -> n11 (serial DVE smalls, ~2us)
            nc.vector.bn_stats(st[:, L - 1, 0:6], xt[:, L - 1, 0:512])
            nc.vector.bn_stats(st[:, L - 1, 6:12], xt[:, L - 1, 512:1024])
            nc.vector.bn_aggr(ag[:, L - 1, :], st[:, L - 1, :])
            vpe11 = spool.tile([P, 1], F32, tag="vpe11")
            nc.vector.tensor_scalar(out=vpe11[:], in0=ag[:, L - 1, 1:2],
                                    scalar1=LN_EPS, scalar2=None, op0=ALU.add)
            r11 = spool.tile([P, 1], F32, tag="r11")
            _rsqrt_newton(nc, spool, vpe11, r11, 1)
            negmur = spool.tile([P, 1], F32, tag="negmur")
            nc.vector.tensor_tensor(out=negmur[:], in0=ag[:, L - 1, 0:1],
                                    in1=r11[:], op=ALU.mult)
            nc.vector.tensor_scalar(out=negmur[:], in0=negmur[:], scalar1=-1.0,
                                    scalar2=None, op0=ALU.mult)
            n11 = npool.tile([P, D], BF16, tag="n11")
            nc.vector.tensor_scalar(
                out=n11[:], in0=xt[:, L - 1, :], scalar1=r11[:],
                scalar2=negmur[:], op0=ALU.mult, op1=ALU.add,
            )
            if use_affine:
                nc.vector.tensor_tensor(out=n11[:], in0=n11[:], in1=lnw_bc[:],
                                        op=ALU.mult)
                nc.vector.tensor_tensor(out=n11[:], in0=n11[:], in1=lnb_bc[:],
                                        op=ALU.add)
            # transpose n11 on TensorE (2 PSUM halves), copy to SBUF bf16
            n11t = npool.tile([P, D], BF16, tag="n11t")
            for hh in range(2):
                pt = pTU.tile([P, 512], BF16, tag="pT")
                for c in range(4):
                    cc = hh * 4 + c
                    nc.tensor.transpose(pt[:, c * P:(c + 1) * P],
                                        n11[:, cc * P:(cc + 1) * P], ident[:])
                nc.scalar.copy(n11t[:, hh * 512:(hh + 1) * 512], pt[:])
            # q^T halves: [dk-half 128, pos 128]
            qs = npool.tile([P, 2 * P], BF16, tag="qs")
            pq = pQ.tile([P, 2 * P], F32, tag="pq")
            for h in range(2):
                for c in range(8):
                    nc.tensor.matmul(
                        pq[:, h * P:(h + 1) * P],
                        lhsT=wqt[:, c * DK + h * P: c * DK + (h + 1) * P],
                        rhs=n11t[:, c * P:(c + 1) * P],
                        start=(c == 0), stop=(c == 7),
                    )
            nc.scalar.copy(qs[:], pq[:])
            # u[pos, d] = Wk.T q  (two PSUM halves from the shared pool)
            usb = npool.tile([P, D], BF16, tag="usb")
            c1h = spool.tile([P, 2], F32, tag="c1h")
            for nh in range(2):
                pu = pTU.tile([P, 512], F32, tag="pu")
                for h in range(2):
                    nc.tensor.matmul(
                        pu[:],
                        lhsT=qs[:, h * P:(h + 1) * P],
                        rhs=wk[:, h * D + nh * 512: h * D + (nh + 1) * 512],
                        start=(h == 0), stop=(h == 1),
                    )
                nc.scalar.activation(out=usb[:, nh * 512:(nh + 1) * 512],
                                     in_=pu[:], func=ACTF.Copy,
                                     accum_out=(None if use_affine
                                                else c1h[:, nh:nh + 1]))
            # per-layer stats l=0..10 (dense DVE work covering the PE/ACT
            # round trips above)
            for l in range(L - 1):
                nc.vector.bn_stats(st[:, l, 0:6], xt[:, l, 0:512])
                nc.vector.bn_stats(st[:, l, 6:12], xt[:, l, 512:1024])
                nc.vector.bn_aggr(ag[:, l, :], st[:, l, :])
            c1 = spool.tile([P, 1], F32, tag="c1")
            if use_affine:
                c2 = spool.tile([P, 1], F32, tag="c2")
                prb = dpool.tile([P, D], F32, tag="c2p")
                nc.vector.scalar_tensor_tensor(
                    out=prb[:], in0=usb[:], scalar=0.0, in1=lnb_bc[:],
                    op0=ALU.add, op1=ALU.mult, accum_out=c2[:])
                nc.vector.tensor_tensor(out=usb[:], in0=usb[:], in1=lnw_bc[:],
                                        op=ALU.mult)
                nc.vector.tensor_reduce(out=c1[:], in_=usb[:],
                                        axis=mybir.AxisListType.X, op=ALU.add)
                T[t]["c2"] = c2
            else:
                nc.vector.tensor_reduce(out=c1[:], in_=c1h[:],
                                        axis=mybir.AxisListType.X, op=ALU.add)
            # u' = u - C1/D  (centering absorbs the mean correction)
            negc1d = spool.tile([P, 1], F32, tag="negc1d")
            nc.vector.tensor_scalar(out=negc1d[:], in0=c1[:],
                                    scalar1=-1.0 / D, scalar2=None,
                                    op0=ALU.mult)
            up = npool.tile([P, D], BF16, tag="up")
            nc.vector.tensor_scalar(out=up[:], in0=usb[:], scalar1=1.0,
                                    scalar2=negc1d[:], op0=ALU.mult,
                                    op1=ALU.add)
            T[t]["up"] = up

        def s2_dots(t):
            xt, up = T[t]["xt"], T[t]["up"]
            acol = spool.tile([P, L], F32, tag="acol")
            for l in range(L):
                dmp = dpool.tile([P, D], BF16, tag="dmp")
                nc.vector.affine_mul_reduce(
                    out=dmp[:], accum_out=acol[:, l:l + 1],
                    in0=xt[:, l, :], in1=up[:], scale=1.0, bias=0.0)
            T[t]["acol"] = acol

        def s3_logits(t):
            ag, acol = T[t]["ag"], T[t]["acol"]
            vpe = spool.tile([P, L], F32, tag="vpe")
            nc.vector.tensor_scalar(out=vpe[:], in0=ag[:, :, 1],
                                    scalar1=LN_EPS, scalar2=None, op0=ALU.add)
            rr = spool.tile([P, L], F32, tag="rr")
            _rsqrt_newton(nc, spool, vpe, rr, L)
            lg = spool.tile([P, L], F32, tag="lg")
            nc.vector.tensor_tensor(out=lg[:], in0=acol[:], in1=rr[:],
                                    op=ALU.mult)
            if use_affine:
                mur = spool.tile([P, L], F32, tag="mur")
                nc.vector.tensor_scalar(out=mur[:], in0=rr[:],
                                        scalar1=T[t]["c2"][:], scalar2=None,
                                        op0=ALU.mult)
                nc.vector.tensor_tensor(out=lg[:], in0=lg[:], in1=mur[:],
                                        op=ALU.add)
            negmax = spool.tile([P, 1], F32, tag="negmax")
            nc.vector.tensor_reduce(out=negmax[:], in_=lg[:],
                                    axis=mybir.AxisListType.X, op=ALU.max,
                                    negate=True)
            nc.vector.tensor_scalar(out=negmax[:], in0=negmax[:], scalar1=SCALE,
                                    scalar2=None, op0=ALU.mult)
            wts = spool.tile([P, L], F32, tag="wts")
            ssum = spool.tile([P, 1], F32, tag="ssum")
            nc.scalar.activation(
                out=wts[:], in_=lg[:], func=ACTF.Exp, bias=negmax[:], scale=SCALE,
                accum_out=ssum[:],
            )
            T[t]["wts"], T[t]["ssum"] = wts, ssum

        def s4_mix(t):
            r0 = t * P
            xt, wts, ssum = T[t]["xt"], T[t]["wts"], T[t]["ssum"]
            rs = spool.tile([P, 1], F32, tag="rs")
            nc.vector.reciprocal(rs[:], ssum[:])
            nc.vector.tensor_scalar(out=rs[:], in0=rs[:], scalar1=(1.0 - g),
                                    scalar2=None, op0=ALU.mult)
            nc.vector.tensor_scalar(out=wts[:], in0=wts[:], scalar1=rs[:],
                                    scalar2=None, op0=ALU.mult)
            nc.vector.tensor_scalar(out=wts[:, L - 1:L], in0=wts[:, L - 1:L],
                                    scalar1=g, scalar2=None, op0=ALU.add)
            pm = pM.tile([P, D], F32, tag="pm")
            for l in range(L):
                dg = dgpool.tile([P, P], BF16, tag="dg")
                nc.vector.tensor_scalar(out=dg[:], in0=ident[:],
                                        scalar1=wts[:, l:l + 1], scalar2=None,
                                        op0=ALU.mult)
                for nh in range(2):
                    nc.tensor.matmul(
                        pm[:, nh * 512:(nh + 1) * 512],
                        lhsT=dg[:],
                        rhs=xt[:, l, nh * 512:(nh + 1) * 512],
                        start=(l == 0), stop=(l == L - 1),
                    )
            osb = opool.tile([P, D], F32, tag="osb")
            nc.scalar.copy(osb[:], pm[:])
            nc.sync.dma_start(out_dram[r0:r0 + P, :], osb[:])
            T[t].clear()

        stages = [s0_load, s1_phase_a, s2_dots, s3_logits, s4_mix]
        nstage = len(stages)
        for step in range(nt + nstage - 1):
            # older tiles' later stages first
            for s in range(nstage - 1, -1, -1):
                t = step - s
                if 0 <= t < nt:
                    stages[s](t)

    nc.compile()
    return nc


_PROGRAM_CACHE = {}


def _get_program(npc, gate, use_affine):
    key = (npc, round(float(gate), 10), bool(use_affine))
    if key not in _PROGRAM_CACHE:
        _PROGRAM_CACHE[key] = build_program(npc, gate, use_affine)
    return _PROGRAM_CACHE[key]


def prep_weights(Wq, Wk):
    """Host-side prep of the replicated small params (bf16 chunk layouts)."""
    bf = mybir.dt.np(BF16)
    wqt = np.ascontiguousarray(
        Wq.T.reshape(8, P, DK).transpose(1, 0, 2).reshape(P, 8 * DK)).astype(bf)
    wkr = np.ascontiguousarray(
        Wk.reshape(2, P, D).transpose(1, 0, 2).reshape(P, 2 * D)).astype(bf)
    return wqt, wkr


def prep_states(states):
    """[L,B,S,D] f32 -> position-major [NTOT, L, D] contiguous."""
    xs = np.asarray(states, dtype=np.float32).reshape(L, NTOT, D)
    return np.ascontiguousarray(xs.transpose(1, 0, 2))


def kernel(states, Wq, Wk, ln_weight, ln_bias, latest_gate, **_unused):
    Wq = np.asarray(Wq, dtype=np.float32)
    Wk = np.asarray(Wk, dtype=np.float32)
    ln_weight = np.asarray(ln_weight, dtype=np.float32)
    ln_bias = np.asarray(ln_bias, dtype=np.float32)
    gate = 1.0 / (1.0 + math.exp(-float(np.asarray(latest_gate))))

    use_affine = not (np.all(ln_weight == 1.0) and np.all(ln_bias == 0.0))
    nc = _get_program(NPC, gate, use_affine)

    wqt, wkr = prep_weights(Wq, Wk)
    xp = prep_states(states)

    in_maps = []
    for c in range(N_CORES):
        m = {
            "states_shard": np.ascontiguousarray(xp[c * NPC:(c + 1) * NPC]),
            "wqt": wqt,
            "wk": wkr,
        }
        if use_affine:
            m["lnw"] = ln_weight.reshape(1, D)
            m["lnb"] = ln_bias.reshape(1, D)
        in_maps.append(m)

    res = run_bass_kernel_spmd(nc, in_maps, list(range(N_CORES)))
    out = np.concatenate([res.results[c]["out"] for c in range(N_CORES)], axis=0)
    return np.ascontiguousarray(out.reshape(B, S, D).astype(np.float32))
